# revision 2
# baseline (speedup 1.0000x reference)
"""DeepGMM Trainium2 kernel — mk-parallel over 8 NeuronCores.

Math: out[b,m,k] = w_mk * (-0.5*(quad + D*log2pi) - logdet_mk), where
quad = ||L^-1 (f_b - mu)||^2, f = relu(x@W+b).
Let A' = sqrt(0.5 w) L^-1, c' = A' mu, z = A' f.
quad' = 0.5 w quad = ||z - c'||^2 = S1 + yneg + kappa,
  S1 = sum z^2, yneg = <f, h'> (h' = -2 A'^T c'), kappa = ||c'||^2.
out = gamma - (S1 + yneg), gamma = beta - kappa,
  beta = -0.5 w D log2pi - w logdet.

Each core handles 10 of the 80 (m,k) pairs with full batch B=4096.
Device: feats GEMM (bf16) -> per pair z-GEMM streaming [A'^T | h'] (the
extra column gives yneg for free) -> square+reduce on ACT/DVE.
"""
import sys
import types

sys.path.insert(0, "/opt/trn_rl_repo")


def _install_ntff_shim():
    # The axon boot looks for antenv.axon_hooks to register its NTFF
    # profiling hook; this image's antenv lacks the module, so provide it.
    if "antenv.axon_hooks" in sys.modules:
        return
    mod = types.ModuleType("antenv.axon_hooks")
    holder = [None]
    mod.set_axon_ntff_profile_hook = lambda h: holder.__setitem__(0, h)
    mod.get_axon_ntff_profile_hook = lambda: holder[0]
    sys.modules["antenv.axon_hooks"] = mod
    try:
        import antenv
        antenv.axon_hooks = mod
    except ImportError:
        pass
    # boot() ran before this shim existed, so its hook registration was
    # skipped; redo it against the real axon .so if present.
    try:
        from trn_agent_boot.trn_boot import _ntff_profile_via_ctypes
        hook = _ntff_profile_via_ctypes("/opt/axon/libaxon_pjrt.so")
        if hook is not None:
            mod.set_axon_ntff_profile_hook(hook)
    except Exception:
        pass


_install_ntff_shim()

import numpy as np
import ml_dtypes

B, D_IN, D_F = 4096, 1024, 256
M, K = 10, 8
NPAIR = 80
NCORE = 8
PPC = NPAIR // NCORE  # pairs per core
LOG2PI = float(np.log(2.0 * np.pi))
BF16 = ml_dtypes.bfloat16

_cache = {}


def _build_module():
    import concourse.bass as bass
    import concourse.tile as tile
    import concourse.mybir as mybir
    from concourse import bacc

    dt = mybir.dt
    AF = mybir.ActivationFunctionType
    ALU = mybir.AluOpType

    nc = bacc.Bacc("TRN2", target_bir_lowering=False, debug=False,
                   enable_asserts=False, num_devices=NCORE)

    x_d = nc.dram_tensor("x_in", [128, 8, B], dt.bfloat16, kind="ExternalInput").ap()
    w_d = nc.dram_tensor("w_in", [128, 8, D_F], dt.bfloat16, kind="ExternalInput").ap()
    b_d = nc.dram_tensor("b_in", [128, 2], dt.float32, kind="ExternalInput").ap()
    rhs_d = nc.dram_tensor("rhs_in", [128, PPC, 2, 257], dt.bfloat16,
                           kind="ExternalInput").ap()
    gam_d = nc.dram_tensor("gam_in", [1, PPC], dt.float32, kind="ExternalInput").ap()
    out_d = nc.dram_tensor("out", [B, PPC], dt.float32, kind="ExternalOutput").ap()

    NB = B // 128  # 32 b-blocks
    NC_CH = 8      # x chunks of 512
    GROUPS = [(0, 4), (4, 4), (8, 2)]

    with tile.TileContext(nc) as tc:
        with (
            tc.tile_pool(name="const", bufs=1) as constp,
            tc.tile_pool(name="xin", bufs=3) as xp,
            tc.tile_pool(name="feat", bufs=1) as fp,
            tc.tile_pool(name="junk", bufs=4) as jp,
            tc.tile_pool(name="stat", bufs=4) as sp,
            tc.tile_pool(name="outp", bufs=4) as op,
            tc.tile_pool(name="ps", bufs=2, space="PSUM") as pp,
        ):
            w_sb = constp.tile([128, 8, D_F], dt.bfloat16)
            nc.sync.dma_start(w_sb[:], w_d[:])
            b_sb = constp.tile([128, 2], dt.float32)
            nc.sync.dma_start(b_sb[:], b_d[:])
            rhs_sb = constp.tile([128, PPC, 2, 257], dt.bfloat16)
            nc.sync.dma_start(rhs_sb[:], rhs_d[:])
            gam_sb = constp.tile([1, PPC], dt.float32)
            nc.sync.dma_start(gam_sb[:], gam_d[:])
            ones_sb = constp.tile([1, 128], dt.bfloat16)
            nc.vector.memset(ones_sb[:], 1.0)
            gam_bf = constp.tile([1, PPC], dt.bfloat16)
            nc.vector.tensor_copy(gam_bf[:], gam_sb[:])

            # gamma broadcast [128, PPC] via ones (x) gamma outer product
            gps = pp.tile([128, 512], dt.float32, tag="ps")
            nc.tensor.matmul(gps[:, 0:PPC], lhsT=ones_sb[:], rhs=gam_bf[:],
                             start=True, stop=True)
            gbc = constp.tile([128, PPC], dt.float32)
            nc.vector.tensor_copy(gbc[:], gps[:, 0:PPC])

            # Phase A: featsT (bf16) [2 fblocks][8 chunks of 512]
            fts = [[None] * NC_CH for _ in range(2)]
            for ch in range(NC_CH):
                xc = xp.tile([128, 8, 512], dt.bfloat16, tag="xc")
                nc.sync.dma_start(xc[:], x_d[:, :, ch * 512:(ch + 1) * 512])
                for fb in range(2):
                    ps = pp.tile([128, 512], dt.float32, tag="ps")
                    for kb in range(8):
                        nc.tensor.matmul(
                            ps[:], lhsT=w_sb[:, kb, fb * 128:(fb + 1) * 128],
                            rhs=xc[:, kb, :], start=(kb == 0), stop=(kb == 7))
                    ft = fp.tile([128, 512], dt.bfloat16, tag=f"ft{fb}_{ch}")
                    nc.scalar.activation(ft[:], ps[:], AF.Relu,
                                         bias=b_sb[:, fb:fb + 1])
                    fts[fb][ch] = ft

            # Phase B: per b-block, per pair z-GEMM + square-reduce
            for bb in range(NB):
                ch, off = bb // 4, (bb % 4) * 128
                l1 = fts[0][ch][:, off:off + 128]
                l2 = fts[1][ch][:, off:off + 128]
                pre = sp.tile([128, PPC], dt.float32, tag="pre")
                for (p0, npair) in GROUPS:
                    pz = pp.tile([128, 2048], dt.float32, tag="ps")
                    s1g = sp.tile([128, npair], dt.float32, tag="s1g")
                    for s in range(npair):
                        p = p0 + s
                        sl = pz[:, s * 512:s * 512 + 257]
                        nc.tensor.matmul(sl, lhsT=l1, rhs=rhs_sb[:, p, 0, :],
                                         start=True, stop=True)
                        nc.tensor.matmul(pz[:, s * 512 + 128:s * 512 + 257],
                                         lhsT=l2, rhs=rhs_sb[:, p, 1, 128:257],
                                         start=False, stop=True,
                                         skip_group_check=True)
                    for s in range(npair):
                        zsl = pz[:, s * 512:s * 512 + 256]
                        if True:  # ACT square path (DVE can't 2x-read PSUM)
                            jt = jp.tile([128, 256], dt.bfloat16, tag="ja")
                            nc.scalar.activation(jt[:], zsl, AF.Square,
                                                 accum_out=s1g[:, s:s + 1])
                        else:
                            jt = jp.tile([128, 256], dt.bfloat16, tag="jd")
                            nc.vector.tensor_tensor_reduce(
                                out=jt[:], in0=zsl, in1=zsl, scale=1.0,
                                scalar=0.0, op0=ALU.mult, op1=ALU.add,
                                accum_out=s1g[:, s:s + 1])
                    aug = pz[:, 0:npair * 512].rearrange(
                        "p (s x) -> p s x", x=512)[:, :, 256]
                    nc.vector.tensor_tensor(pre[:, p0:p0 + npair], aug,
                                            s1g[:], op=ALU.add)
                ot = op.tile([128, PPC], dt.float32, tag="ot")
                nc.vector.tensor_sub(ot[:], gbc[:], pre[:])
                nc.sync.dma_start(out_d[bb * 128:(bb + 1) * 128, :], ot[:])
    nc.finalize()
    return nc


def _prep_inputs(x, W, b, means, covs, weights):
    # host: shard/cast/layout + small per-pair parameter preprocessing
    x = np.asarray(x, np.float32)
    W = np.asarray(W, np.float32)
    b = np.asarray(b, np.float32)
    means = np.asarray(means, np.float32).reshape(NPAIR, D_F)
    covs = np.asarray(covs, np.float32).reshape(NPAIR, D_F, D_F)
    weights = np.asarray(weights, np.float32)

    x_in = np.ascontiguousarray(
        x.T.reshape(8, 128, B).transpose(1, 0, 2)).astype(BF16)
    w_in = np.ascontiguousarray(
        W.reshape(8, 128, D_F).transpose(1, 0, 2)).astype(BF16)
    b_in = np.ascontiguousarray(b.reshape(2, 128).T).astype(np.float32)

    ew = np.exp(weights - weights.max(axis=1, keepdims=True))
    w_sm = (ew / ew.sum(axis=1, keepdims=True)).reshape(NPAIR)

    from scipy.linalg import solve_triangular
    rhs_all = np.zeros((NPAIR, D_F, 257), np.float32)
    gam_all = np.zeros(NPAIR, np.float32)
    eye = np.eye(D_F, dtype=np.float32)
    for q in range(NPAIR):
        L = np.tril(covs[q])
        A = solve_triangular(L, eye, lower=True)
        s = np.sqrt(0.5 * w_sm[q])
        Ap = s * A
        cp = Ap @ means[q]
        hp = -2.0 * (Ap.T @ cp)
        logdet = np.log(np.diag(L)).sum()
        beta = -0.5 * w_sm[q] * D_F * LOG2PI - w_sm[q] * logdet
        gam_all[q] = beta - float(cp @ cp)
        rhs_all[q, :, 0:256] = Ap.T
        rhs_all[q, :, 256] = hp

    in_maps = []
    for c in range(NCORE):
        sl = slice(c * PPC, (c + 1) * PPC)
        rhs_c = np.ascontiguousarray(
            rhs_all[sl].reshape(PPC, 2, 128, 257).transpose(2, 0, 1, 3)
        ).astype(BF16)
        gam_c = gam_all[sl].reshape(1, PPC).astype(np.float32)
        in_maps.append({
            "x_in": x_in, "w_in": w_in, "b_in": b_in,
            "rhs_in": rhs_c, "gam_in": gam_c,
        })
    return in_maps


def kernel(x, W, b, means, covs, weights, _want_trace=False):
    from concourse import bass_utils

    if "nc" not in _cache:
        _cache["nc"] = _build_module()
    nc = _cache["nc"]
    in_maps = _prep_inputs(x, W, b, means, covs, weights)
    res = bass_utils.run_bass_kernel_spmd(
        nc, in_maps, core_ids=list(range(NCORE)), trace=_want_trace)
    if _want_trace:
        _cache["last_results"] = res
    out = np.concatenate([res.results[c]["out"] for c in range(NCORE)],
                         axis=1)
    return np.ascontiguousarray(out.reshape(B, M, K).astype(np.float32))



# revision 4
# speedup vs baseline: 1.0348x; 1.0348x over previous
"""DeepGMM Trainium2 kernel — mk-parallel over 8 NeuronCores.

Math: out[b,m,k] = w_mk * (-0.5*(quad + D*log2pi) - logdet_mk), where
quad = ||L^-1 (f_b - mu)||^2, f = relu(x@W+b).
Let A' = sqrt(0.5 w) L^-1, c' = A' mu, z = A' f.
quad' = 0.5 w quad = ||z - c'||^2 = S1 + yneg + kappa,
  S1 = sum z^2, yneg = <f, h'> (h' = -2 A'^T c'), kappa = ||c'||^2.
out = gamma - (S1 + yneg), gamma = beta - kappa,
  beta = -0.5 w D log2pi - w logdet.

Each core handles 10 of the 80 (m,k) pairs with full batch B=4096.
Device: feats GEMM (bf16) -> per pair z-GEMM streaming [A'^T | h'] (the
extra column gives yneg for free) -> square+reduce on ACT/DVE.
"""
import sys
import types

sys.path.insert(0, "/opt/trn_rl_repo")


def _install_ntff_shim():
    # The axon boot looks for antenv.axon_hooks to register its NTFF
    # profiling hook; this image's antenv lacks the module, so provide it.
    if "antenv.axon_hooks" in sys.modules:
        return
    mod = types.ModuleType("antenv.axon_hooks")
    holder = [None]
    mod.set_axon_ntff_profile_hook = lambda h: holder.__setitem__(0, h)
    mod.get_axon_ntff_profile_hook = lambda: holder[0]
    sys.modules["antenv.axon_hooks"] = mod
    try:
        import antenv
        antenv.axon_hooks = mod
    except ImportError:
        pass
    # boot() ran before this shim existed, so its hook registration was
    # skipped; redo it against the real axon .so if present.
    try:
        from trn_agent_boot.trn_boot import _ntff_profile_via_ctypes
        hook = _ntff_profile_via_ctypes("/opt/axon/libaxon_pjrt.so")
        if hook is not None:
            mod.set_axon_ntff_profile_hook(hook)
    except Exception:
        pass


_install_ntff_shim()

import numpy as np
import ml_dtypes

B, D_IN, D_F = 4096, 1024, 256
M, K = 10, 8
NPAIR = 80
NCORE = 8
PPC = NPAIR // NCORE  # pairs per core
LOG2PI = float(np.log(2.0 * np.pi))
BF16 = ml_dtypes.bfloat16

_cache = {}


def _build_module():
    import concourse.bass as bass
    import concourse.tile as tile
    import concourse.mybir as mybir
    from concourse import bacc

    dt = mybir.dt
    AF = mybir.ActivationFunctionType
    ALU = mybir.AluOpType
    AXX = mybir.AxisListType.X

    nc = bacc.Bacc("TRN2", target_bir_lowering=False, debug=False,
                   enable_asserts=False, num_devices=NCORE)

    x_d = nc.dram_tensor("x_in", [128, 8, B], dt.bfloat16, kind="ExternalInput").ap()
    w_d = nc.dram_tensor("w_in", [128, 8, D_F], dt.bfloat16, kind="ExternalInput").ap()
    b_d = nc.dram_tensor("b_in", [128, 2], dt.float32, kind="ExternalInput").ap()
    rhs_d = nc.dram_tensor("rhs_in", [128, PPC, 2, 257], dt.bfloat16,
                           kind="ExternalInput").ap()
    gam_d = nc.dram_tensor("gam_in", [1, PPC], dt.float32, kind="ExternalInput").ap()
    out_d = nc.dram_tensor("out", [B, PPC], dt.float32, kind="ExternalOutput").ap()

    NB = B // 128  # 32 b-blocks
    NC_CH = 8      # x chunks of 512
    GROUPS = [(0, 4), (4, 4), (8, 2)]

    with tile.TileContext(nc) as tc:
        with (
            tc.tile_pool(name="const", bufs=1) as constp,
            tc.tile_pool(name="xin", bufs=3) as xp,
            tc.tile_pool(name="feat", bufs=1) as fp,
            tc.tile_pool(name="junk", bufs=4) as jp,
            tc.tile_pool(name="stat", bufs=4) as sp,
            tc.tile_pool(name="outp", bufs=4) as op,
            tc.tile_pool(name="ps", bufs=2, space="PSUM") as pp,
        ):
            w_sb = constp.tile([128, 8, D_F], dt.bfloat16)
            nc.sync.dma_start(w_sb[:], w_d[:])
            b_sb = constp.tile([128, 2], dt.float32)
            nc.sync.dma_start(b_sb[:], b_d[:])
            rhs_sb = constp.tile([128, PPC, 2, 257], dt.bfloat16)
            nc.sync.dma_start(rhs_sb[:], rhs_d[:])
            gam_sb = constp.tile([1, PPC], dt.float32)
            nc.sync.dma_start(gam_sb[:], gam_d[:])
            ones_sb = constp.tile([1, 128], dt.bfloat16)
            nc.vector.memset(ones_sb[:], 1.0)
            gam_bf = constp.tile([1, PPC], dt.bfloat16)
            nc.vector.tensor_copy(gam_bf[:], gam_sb[:])

            # gamma broadcast [128, PPC] via ones (x) gamma outer product
            gps = pp.tile([128, 512], dt.float32, tag="ps")
            nc.tensor.matmul(gps[:, 0:PPC], lhsT=ones_sb[:], rhs=gam_bf[:],
                             start=True, stop=True)
            gbc = constp.tile([128, PPC], dt.float32)
            nc.vector.tensor_copy(gbc[:], gps[:, 0:PPC])

            # Phase A: featsT (bf16) [2 fblocks][8 chunks of 512]
            fts = [[None] * NC_CH for _ in range(2)]
            for ch in range(NC_CH):
                xc = xp.tile([128, 8, 512], dt.bfloat16, tag="xc")
                nc.sync.dma_start(xc[:], x_d[:, :, ch * 512:(ch + 1) * 512])
                for fb in range(2):
                    ps = pp.tile([128, 512], dt.float32, tag="ps")
                    for kb in range(8):
                        nc.tensor.matmul(
                            ps[:], lhsT=w_sb[:, kb, fb * 128:(fb + 1) * 128],
                            rhs=xc[:, kb, :], start=(kb == 0), stop=(kb == 7))
                    ft = fp.tile([128, 512], dt.bfloat16, tag=f"ft{fb}_{ch}")
                    nc.scalar.activation(ft[:], ps[:], AF.Relu,
                                         bias=b_sb[:, fb:fb + 1])
                    fts[fb][ch] = ft

            # Phase B: per b-block, per pair z-GEMM + square-reduce
            for bb in range(NB):
                ch, off = bb // 4, (bb % 4) * 128
                l1 = fts[0][ch][:, off:off + 128]
                l2 = fts[1][ch][:, off:off + 128]
                pre = sp.tile([128, PPC], dt.float32, tag="pre")
                for (p0, npair) in GROUPS:
                    pz = pp.tile([128, 2048], dt.float32, tag="ps")
                    s1g = sp.tile([128, npair], dt.float32, tag="s1g")
                    # batch same-stationary matmuls to halve LDWEIGHTS
                    for s in range(npair):
                        p = p0 + s
                        sl = pz[:, s * 512:s * 512 + 257]
                        nc.tensor.matmul(sl, lhsT=l1, rhs=rhs_sb[:, p, 0, :],
                                         start=True, stop=True)
                    for s in range(npair):
                        p = p0 + s
                        nc.tensor.matmul(pz[:, s * 512 + 128:s * 512 + 257],
                                         lhsT=l2, rhs=rhs_sb[:, p, 1, 128:257],
                                         start=False, stop=True,
                                         skip_group_check=True)
                    if npair > 2:
                        # grouped ACT square (no accumulator read) + DVE
                        # segmented reduce over the innermost axis
                        zg = pz[:, 0:npair * 512].rearrange(
                            "p (s x) -> p s x", x=512)[:, :, 0:256]
                        jt = jp.tile([128, npair, 256], dt.bfloat16, tag="ja")
                        nc.scalar.activation(jt[:], zg, AF.Square)
                        nc.vector.tensor_reduce(
                            out=s1g[:], in_=jt[:], axis=AXX, op=ALU.add)
                    else:
                        for s in range(npair):
                            zsl = pz[:, s * 512:s * 512 + 256]
                            jt = jp.tile([128, 256], dt.bfloat16, tag="jf")
                            nc.scalar.activation(jt[:], zsl, AF.Square,
                                                 accum_out=s1g[:, s:s + 1])
                    aug = pz[:, 0:npair * 512].rearrange(
                        "p (s x) -> p s x", x=512)[:, :, 256]
                    nc.vector.tensor_tensor(pre[:, p0:p0 + npair], aug,
                                            s1g[:], op=ALU.add)
                ot = op.tile([128, PPC], dt.float32, tag="ot")
                nc.vector.tensor_sub(ot[:], gbc[:], pre[:])
                nc.sync.dma_start(out_d[bb * 128:(bb + 1) * 128, :], ot[:])
    nc.finalize()
    return nc


def _prep_inputs(x, W, b, means, covs, weights):
    # host: shard/cast/layout + small per-pair parameter preprocessing
    x = np.asarray(x, np.float32)
    W = np.asarray(W, np.float32)
    b = np.asarray(b, np.float32)
    means = np.asarray(means, np.float32).reshape(NPAIR, D_F)
    covs = np.asarray(covs, np.float32).reshape(NPAIR, D_F, D_F)
    weights = np.asarray(weights, np.float32)

    x_in = np.ascontiguousarray(
        x.T.reshape(8, 128, B).transpose(1, 0, 2)).astype(BF16)
    w_in = np.ascontiguousarray(
        W.reshape(8, 128, D_F).transpose(1, 0, 2)).astype(BF16)
    b_in = np.ascontiguousarray(b.reshape(2, 128).T).astype(np.float32)

    ew = np.exp(weights - weights.max(axis=1, keepdims=True))
    w_sm = (ew / ew.sum(axis=1, keepdims=True)).reshape(NPAIR)

    from scipy.linalg import solve_triangular
    rhs_all = np.zeros((NPAIR, D_F, 257), np.float32)
    gam_all = np.zeros(NPAIR, np.float32)
    eye = np.eye(D_F, dtype=np.float32)
    for q in range(NPAIR):
        L = np.tril(covs[q])
        A = solve_triangular(L, eye, lower=True)
        s = np.sqrt(0.5 * w_sm[q])
        Ap = s * A
        cp = Ap @ means[q]
        hp = -2.0 * (Ap.T @ cp)
        logdet = np.log(np.diag(L)).sum()
        beta = -0.5 * w_sm[q] * D_F * LOG2PI - w_sm[q] * logdet
        gam_all[q] = beta - float(cp @ cp)
        rhs_all[q, :, 0:256] = Ap.T
        rhs_all[q, :, 256] = hp

    in_maps = []
    for c in range(NCORE):
        sl = slice(c * PPC, (c + 1) * PPC)
        rhs_c = np.ascontiguousarray(
            rhs_all[sl].reshape(PPC, 2, 128, 257).transpose(2, 0, 1, 3)
        ).astype(BF16)
        gam_c = gam_all[sl].reshape(1, PPC).astype(np.float32)
        in_maps.append({
            "x_in": x_in, "w_in": w_in, "b_in": b_in,
            "rhs_in": rhs_c, "gam_in": gam_c,
        })
    return in_maps


def kernel(x, W, b, means, covs, weights, _want_trace=False):
    from concourse import bass_utils

    if "nc" not in _cache:
        _cache["nc"] = _build_module()
    nc = _cache["nc"]
    in_maps = _prep_inputs(x, W, b, means, covs, weights)
    res = bass_utils.run_bass_kernel_spmd(
        nc, in_maps, core_ids=list(range(NCORE)), trace=_want_trace)
    if _want_trace:
        _cache["last_results"] = res
    out = np.concatenate([res.results[c]["out"] for c in range(NCORE)],
                         axis=1)
    return np.ascontiguousarray(out.reshape(B, M, K).astype(np.float32))

